# revision 19
# baseline (speedup 1.0000x reference)
"""DeepseekMoE block-quantized MoE kernel for 8 Trainium2 NeuronCores.

Strategy (expert-parallel with host-side dispatch):
  - The routing table (selected_experts) is known on the host before launch,
    so the all-to-all "dispatch" is done on the host: for each expert e we
    gather the unique tokens routed to it (dedup across the top-k slots),
    transpose to [H, n_e], and pad to a common capacity C.
  - Experts are sharded 2-per-core across the 8 cores.  Each core runs a
    dense 3-matmul MLP (gate/up -> silu*up -> down) for its 2 experts in
    x^T / act^T layout so no on-device transposes are needed.
  - Block-dequantization (w * repeat(s, 128)) is folded into the host-side
    weight preparation.
  - All tensors are bf16 on device (PSUM accumulation stays fp32): same
    1 col/cycle PE streaming rate as float32r, but half the DMA bytes and
    SBUF footprint, and LDWEIGHTS gets fast-weight-load (2x).
  - Weights are stored in DRAM in slab-major layout ([slab, 128, free])
    so each weight-slab DMA is 128 descriptors of 4KB/2.8KB contiguous
    instead of 2048 descriptors of 512B (the DMA engines are
    descriptor-rate-bound, not byte-bound).
  - The host scatters the per-expert outputs back to [T, K, H].
"""

import math

import numpy as np

T = 4096
TOPK = 6
E = 16
H = 2048
I = 1408
BS = 128           # quant block size
HT = H // 128      # 16 h-tiles
IT = I // 128      # 11 i-tiles
NCORES = 8
# Single-pass SBUF budget bound (bf16): (HT + IT) * 2 * W bytes of x+act per
# partition plus ~35KB of weight/output staging must fit in ~208KB.
MAX_W = 2944

_BUILT = {}
LAST_RESULTS = None  # stashed BassKernelResults for external harnesses


def _chunk_plan(width):
    """Split `width` columns into PSUM-bank-sized chunks (<=512), each >=256
    when width allows (keeps every matmul well above the LDWEIGHTS shadow)."""
    if width <= 512:
        return [(0, width)]
    n = -(-width // 512)
    # use 8-aligned chunk widths
    base = (width // n) // 8 * 8
    rem8 = (width - n * base) // 8
    out, off = [], 0
    for j in range(n):
        w = base + (8 if j < rem8 else 0)
        if j == n - 1:
            w = width - off
        out.append((off, w))
        off += w
    return out


def _build(jobs, CT):
    """Build the SPMD Bass program.  `jobs` is a tuple of
    (slot, col_offset, width): each job runs one expert slot's MLP over a
    window of `width` token columns; CT is the column capacity of xt/yt."""
    import concourse.bacc as bacc
    import concourse.mybir as mybir
    from concourse.bass import ts
    from concourse.tile import TileContext

    f32 = mybir.dt.float32
    bf16 = mybir.dt.bfloat16
    AF = mybir.ActivationFunctionType
    import os as _os

    act_fn = (
        AF.Sigmoid if _os.environ.get("KERNEL_SIM_SIGMOID") else AF.Silu
    )  # CoreSim lacks Silu; HW path always uses Silu

    nc = bacc.Bacc()
    xt = nc.declare_dram_parameter("xt", [2, HT, 128, CT], bf16, isOutput=False)
    # slab-major weights: w0t/w1t[s, it, p, hb*128+j] = Wdq[it*128+j, hb*128+p]
    w0t = nc.declare_dram_parameter("w0t", [2, IT, 128, H], bf16, isOutput=False)
    w1t = nc.declare_dram_parameter("w1t", [2, IT, 128, H], bf16, isOutput=False)
    # w2t[s, ht, p, it*128+j] = W2dq[ht*128+j, it*128+p]
    w2t = nc.declare_dram_parameter("w2t", [2, HT, 128, I], bf16, isOutput=False)
    yt = nc.declare_dram_parameter("yt", [2, HT, 128, CT], bf16, isOutput=True)

    with TileContext(nc) as tc:
        with (
            tc.tile_pool(name="xp", bufs=1) as xp,
            tc.tile_pool(name="ap", bufs=1) as apool,
            tc.tile_pool(name="wp", bufs=2) as wp,
            tc.tile_pool(name="yp", bufs=10) as yp,
            tc.tile_pool(name="ps", bufs=2, space="PSUM") as ps,
        ):
            # PE warmup: dummy matmuls on scratch tiles while the first x/w
            # DMAs stream in, so the HAM clock gate reaches 2.4 GHz before
            # real matmuls start (otherwise the first ~3.4us run at 1.2 GHz).
            warm_sb = xp.tile([128, 192], bf16, tag="warm")
            nc.vector.memset(warm_sb, 0.0)
            warm_ps = ps.tile([128, 512], f32, tag="o", bufs=4)
            for _ in range(64):
                nc.tensor.matmul(
                    warm_ps[:, :192], warm_sb[:, :128], warm_sb,
                    start=True, stop=True,
                )

            for jn, (s, co, W) in enumerate(jobs):
                    chunks = _chunk_plan(W)
                    # one SBUF tile holds all 16 h-tiles of x side by side so
                    # each chunk loads with a single 3D-AP DMA (the ~600ns
                    # per-dma_start issue cost dominates otherwise; packets
                    # of one DMA already round-robin across all 16 engines)
                    xs_all = xp.tile([128, HT * W], bf16, tag="x", name=f"x_{jn}")

                    def load_x_chunk(ci, eng=None):
                        c0, cw = chunks[ci]
                        (eng or nc.sync).dma_start(
                            out=xs_all.rearrange("p (h w) -> p h w", h=HT)[
                                :, :, c0 : c0 + cw
                            ],
                            in_=xt[s, :, :, co + c0 : co + c0 + cw].rearrange(
                                "h p w -> p h w"
                            ),
                        )

                    def xsl(h, c0, cw):
                        return xs_all[:, h * W + c0 : h * W + c0 + cw]

                    acts = [
                        apool.tile([128, W], bf16, tag=f"a{i}", name=f"a{i}_{jn}")
                        for i in range(IT)
                    ]

                    # Phase A: chunk-outer / i-inner, with all 22 w0/w1 slabs
                    # loaded during the first chunk pass and kept resident in
                    # SBUF (88KB/partition).  This spreads the x-chunk DMA
                    # demand over the whole phase instead of front-loading it,
                    # and each slab is still loaded exactly once per job.
                    slabs = {}

                    def load_slab_pair(i, eng=None):
                        w0s = wp.tile([128, H], bf16, tag=f"w0_{i}", bufs=1)
                        (eng or nc.sync).dma_start(out=w0s, in_=w0t[s, i])
                        w1s = wp.tile([128, H], bf16, tag=f"w1_{i}", bufs=1)
                        (eng or nc.sync).dma_start(out=w1s, in_=w1t[s, i])
                        slabs[i] = (w0s, w1s)

                    if jn == 0:
                        # gate loads go via the Scalar engine's DGE queue: its
                        # preamble ends ~1.5us before Sync's, and w0 slab + x
                        # chunk 0 (the first matmul group's inputs) go first
                        w0s = wp.tile([128, H], bf16, tag="w0_0", bufs=1)
                        nc.scalar.dma_start(out=w0s, in_=w0t[s, 0])
                        load_x_chunk(0, eng=nc.scalar)
                        w1s = wp.tile([128, H], bf16, tag="w1_0", bufs=1)
                        nc.scalar.dma_start(out=w1s, in_=w1t[s, 0])
                        slabs[0] = (w0s, w1s)
                    else:
                        load_slab_pair(0)
                        load_x_chunk(0)
                    for ci, (c0, cw) in enumerate(chunks):
                        for i in range(IT):
                            if ci == 0:
                                # prefetch next slab pair one group ahead
                                if i + 1 < IT:
                                    load_slab_pair(i + 1)
                                if i == 2 and len(chunks) > 1:
                                    load_x_chunk(1)
                                if i == 6:
                                    for cj in range(2, len(chunks)):
                                        load_x_chunk(cj)
                            w0s, w1s = slabs[i]
                            g = ps.tile([128, 512], f32, tag="g")
                            for h in range(HT):
                                nc.tensor.matmul(
                                    g[:, :cw],
                                    w0s[:, ts(h, 128)],
                                    xsl(h, c0, cw),
                                    start=(h == 0),
                                    stop=(h == HT - 1),
                                )
                            u = ps.tile([128, 512], f32, tag="u")
                            for h in range(HT):
                                nc.tensor.matmul(
                                    u[:, :cw],
                                    w1s[:, ts(h, 128)],
                                    xsl(h, c0, cw),
                                    start=(h == 0),
                                    stop=(h == HT - 1),
                                )
                            a_sl = acts[i][:, c0 : c0 + cw]
                            nc.scalar.activation(a_sl, g[:, :cw], act_fn)
                            nc.vector.tensor_mul(a_sl, a_sl, u[:, :cw])

                    # Phase B: down projection, per h-tile.
                    for h in range(HT):
                        w2s = wp.tile([128, I], bf16, tag="w2", bufs=3)
                        nc.sync.dma_start(out=w2s, in_=w2t[s, h])
                        yc = yp.tile([128, W], bf16, tag="y", bufs=3)
                        for c0, cw in chunks:
                            o = ps.tile([128, 512], f32, tag="o", bufs=4)
                            for i in range(IT):
                                nc.tensor.matmul(
                                    o[:, :cw],
                                    w2s[:, ts(i, 128)],
                                    acts[i][:, c0 : c0 + cw],
                                    start=(i == 0),
                                    stop=(i == IT - 1),
                                )
                            nc.vector.tensor_copy(yc[:, c0 : c0 + cw], o[:, :cw])
                            if jn == len(jobs) - 1 and h == HT - 1:
                                # final h-tile: per-chunk writes shrink the tail
                                nc.scalar.dma_start(
                                    out=yt[s, h, :, co + c0 : co + c0 + cw],
                                    in_=yc[:, c0 : c0 + cw],
                                )
                        # one batched y write per h-tile, issued from the
                        # Scalar engine's HW-DGE queue so its CAST-waits never
                        # head-of-line block the input loads on the Sync queue.
                        if not (jn == len(jobs) - 1 and h == HT - 1):
                            nc.scalar.dma_start(
                                out=yt[s, h, :, co : co + W], in_=yc[:, :W]
                            )
    nc.finalize()
    return nc


def _get_built(jobs, CT):
    key = (tuple(jobs), CT)
    if key not in _BUILT:
        _BUILT[key] = _build(tuple(jobs), CT)
    return _BUILT[key]


def _dequant(w, s):
    """w: [E, O, Iin], s: [E, O, Iin//128] -> dequantized [E, O, Iin]."""
    e, o, iin = w.shape
    return (w.reshape(e, o, iin // BS, BS) * s[..., None]).reshape(e, o, iin)


def _bf16(a):
    import ml_dtypes

    return np.ascontiguousarray(a.astype(ml_dtypes.bfloat16))


def kernel(**inputs):
    global LAST_RESULTS
    x = np.ascontiguousarray(np.asarray(inputs["x"], dtype=np.float32))
    sel = np.asarray(inputs["selected_experts"])
    w0 = np.asarray(inputs["w0"], dtype=np.float32)
    s0 = np.asarray(inputs["s0"], dtype=np.float32)
    w1 = np.asarray(inputs["w1"], dtype=np.float32)
    s1 = np.asarray(inputs["s1"], dtype=np.float32)
    w2 = np.asarray(inputs["w2"], dtype=np.float32)
    s2 = np.asarray(inputs["s2"], dtype=np.float32)

    t, k = sel.shape
    assert (t, k) == (T, TOPK) and x.shape == (T, H)

    # ---- host-side dispatch: unique tokens per expert ----
    pos = np.full((E, T), -1, dtype=np.int32)
    cols = []
    for e in range(E):
        toks = np.nonzero((sel == e).any(axis=1))[0]
        cols.append(toks)
        pos[e, toks] = np.arange(len(toks), dtype=np.int32)
    counts = np.array([len(c) for c in cols])

    # Assign experts to (core, slot): slot 0 holds the 8 largest experts,
    # slot 1 the 8 smallest, so each slot's padded width is only the max of
    # its own rank group.  expert_of[s][c] = expert on core c, slot s.
    order = np.argsort(-counts, kind="stable")
    expert_of = [list(order[:NCORES]), list(order[NCORES:])]

    def align8(v):
        return max(256, -(-v // 8) * 8)

    slot_w = [align8(int(counts[expert_of[s]].max())) for s in range(2)]

    if max(slot_w) <= MAX_W:
        jobs = tuple((s, 0, slot_w[s]) for s in range(2))
        CT = max(slot_w)
    else:
        # fallback: uniform width, multiple column windows per slot
        cmax = int(counts.max())
        passes = max(1, math.ceil(cmax / MAX_W))
        W = align8(math.ceil(cmax / passes))
        CT = W * passes
        jobs = tuple((s, cp * W, W) for s in range(2) for cp in range(passes))

    # ---- dequantize + slab-major weight prep (host) ----
    W0 = _dequant(w0, s0)  # [E, I, H]
    W1 = _dequant(w1, s1)  # [E, I, H]
    W2 = _dequant(w2, s2)  # [E, H, I]
    # w01 slab-major: [E, IT, 128p, H] with [it, p, hb*128+j] = W[it*128+j, hb*128+p]
    w0n = _bf16(W0.reshape(E, IT, 128, HT, 128).transpose(0, 1, 4, 3, 2).reshape(E, IT, 128, H))
    w1n = _bf16(W1.reshape(E, IT, 128, HT, 128).transpose(0, 1, 4, 3, 2).reshape(E, IT, 128, H))
    # w2 slab-major: [E, HT, 128p, I] with [ht, p, it*128+j] = W2[ht*128+j, it*128+p]
    w2n = _bf16(W2.reshape(E, HT, 128, IT, 128).transpose(0, 1, 4, 3, 2).reshape(E, HT, 128, I))

    in_maps = []
    for c in range(NCORES):
        pair = [expert_of[0][c], expert_of[1][c]]
        xt_c = np.zeros((2, H, CT), dtype=np.float32)
        for s, e in enumerate(pair):
            n = len(cols[e])
            if n:
                xt_c[s, :, :n] = x[cols[e]].T
        in_maps.append(
            {
                "xt": _bf16(xt_c.reshape(2, HT, 128, CT)),
                "w0t": np.ascontiguousarray(w0n[pair]),
                "w1t": np.ascontiguousarray(w1n[pair]),
                "w2t": np.ascontiguousarray(w2n[pair]),
            }
        )

    nc = _get_built(jobs, CT)
    from concourse.bass_utils import run_bass_kernel_spmd

    res = run_bass_kernel_spmd(nc, in_maps, list(range(NCORES)))
    LAST_RESULTS = res

    # Y[e] = [H, CT] for expert e
    Y = np.empty((E, H, CT), dtype=np.float32)
    for c in range(NCORES):
        yt_c = np.asarray(res.results[c]["yt"]).astype(np.float32).reshape(2, H, CT)
        Y[expert_of[0][c]] = yt_c[0]
        Y[expert_of[1][c]] = yt_c[1]

    # ---- scatter back to [T, K, H] ----
    e_flat = sel.reshape(-1).astype(np.int64)
    t_flat = np.repeat(np.arange(T, dtype=np.int64), TOPK)
    p_flat = pos[e_flat, t_flat]
    out = Y[e_flat, :, p_flat]  # [T*K, H]
    return np.ascontiguousarray(out.reshape(T, TOPK, H), dtype=np.float32)


# revision 21
# speedup vs baseline: 1.0210x; 1.0210x over previous
"""DeepseekMoE block-quantized MoE kernel for 8 Trainium2 NeuronCores.

Strategy (expert-parallel with host-side dispatch):
  - The routing table (selected_experts) is known on the host before launch,
    so the all-to-all "dispatch" is done on the host: for each expert e we
    gather the unique tokens routed to it (dedup across the top-k slots),
    transpose to [H, n_e], and pad to a common capacity C.
  - Experts are sharded 2-per-core across the 8 cores.  Each core runs a
    dense 3-matmul MLP (gate/up -> silu*up -> down) for its 2 experts in
    x^T / act^T layout so no on-device transposes are needed.
  - Block-dequantization (w * repeat(s, 128)) is folded into the host-side
    weight preparation.
  - All tensors are bf16 on device (PSUM accumulation stays fp32): same
    1 col/cycle PE streaming rate as float32r, but half the DMA bytes and
    SBUF footprint, and LDWEIGHTS gets fast-weight-load (2x).
  - Weights are stored in DRAM in slab-major layout ([slab, 128, free])
    so each weight-slab DMA is 128 descriptors of 4KB/2.8KB contiguous
    instead of 2048 descriptors of 512B (the DMA engines are
    descriptor-rate-bound, not byte-bound).
  - The host scatters the per-expert outputs back to [T, K, H].
"""

import math

import numpy as np

T = 4096
TOPK = 6
E = 16
H = 2048
I = 1408
BS = 128           # quant block size
HT = H // 128      # 16 h-tiles
IT = I // 128      # 11 i-tiles
NCORES = 8
# Single-pass SBUF budget bound (bf16): (HT + IT) * 2 * W bytes of x+act per
# partition plus ~35KB of weight/output staging must fit in ~208KB.
MAX_W = 2944

_BUILT = {}
LAST_RESULTS = None  # stashed BassKernelResults for external harnesses


def _chunk_plan(width):
    """Split `width` columns into PSUM-bank-sized chunks (<=512), each >=256
    when width allows (keeps every matmul well above the LDWEIGHTS shadow)."""
    if width <= 512:
        return [(0, width)]
    n = -(-width // 512)
    # use 8-aligned chunk widths
    base = (width // n) // 8 * 8
    rem8 = (width - n * base) // 8
    out, off = [], 0
    for j in range(n):
        w = base + (8 if j < rem8 else 0)
        if j == n - 1:
            w = width - off
        out.append((off, w))
        off += w
    return out


def _build(jobs, CT):
    """Build the SPMD Bass program.  `jobs` is a tuple of
    (slot, col_offset, width): each job runs one expert slot's MLP over a
    window of `width` token columns; CT is the column capacity of xt/yt."""
    import concourse.bacc as bacc
    import concourse.mybir as mybir
    from concourse.bass import ts
    from concourse.tile import TileContext

    f32 = mybir.dt.float32
    bf16 = mybir.dt.bfloat16
    AF = mybir.ActivationFunctionType
    import os as _os

    act_fn = (
        AF.Sigmoid if _os.environ.get("KERNEL_SIM_SIGMOID") else AF.Silu
    )  # CoreSim lacks Silu; HW path always uses Silu

    nc = bacc.Bacc()
    xt = nc.declare_dram_parameter("xt", [2, HT, 128, CT], bf16, isOutput=False)
    # slab-major weights: w0t/w1t[s, it, p, hb*128+j] = Wdq[it*128+j, hb*128+p]
    w0t = nc.declare_dram_parameter("w0t", [2, IT, 128, H], bf16, isOutput=False)
    w1t = nc.declare_dram_parameter("w1t", [2, IT, 128, H], bf16, isOutput=False)
    # w2t[s, ht, p, it*128+j] = W2dq[ht*128+j, it*128+p]
    w2t = nc.declare_dram_parameter("w2t", [2, HT, 128, I], bf16, isOutput=False)
    yt = nc.declare_dram_parameter("yt", [2, HT, 128, CT], bf16, isOutput=True)

    with TileContext(nc) as tc:
        with (
            tc.tile_pool(name="xp", bufs=1) as xp,
            tc.tile_pool(name="ap", bufs=1) as apool,
            tc.tile_pool(name="wp", bufs=2) as wp,
            tc.tile_pool(name="yp", bufs=10) as yp,
            tc.tile_pool(name="ps", bufs=2, space="PSUM") as ps,
        ):
            # PE warmup: dummy matmuls on scratch tiles while the first x/w
            # DMAs stream in, so the HAM clock gate reaches 2.4 GHz before
            # real matmuls start (otherwise the first ~3.4us run at 1.2 GHz).
            warm_sb = xp.tile([128, 192], bf16, tag="warm")
            nc.vector.memset(warm_sb, 0.0)
            warm_ps = ps.tile([128, 512], f32, tag="o", bufs=4)
            for _ in range(100):
                nc.tensor.matmul(
                    warm_ps[:, :192], warm_sb[:, :128], warm_sb,
                    start=True, stop=True,
                )

            for jn, (s, co, W) in enumerate(jobs):
                    chunks = _chunk_plan(W)
                    # one SBUF tile holds all 16 h-tiles of x side by side so
                    # each chunk loads with a single 3D-AP DMA (the ~600ns
                    # per-dma_start issue cost dominates otherwise; packets
                    # of one DMA already round-robin across all 16 engines)
                    xs_all = xp.tile([128, HT * W], bf16, tag="x", name=f"x_{jn}")

                    def load_x_chunk(ci, eng=None):
                        c0, cw = chunks[ci]
                        (eng or nc.sync).dma_start(
                            out=xs_all.rearrange("p (h w) -> p h w", h=HT)[
                                :, :, c0 : c0 + cw
                            ],
                            in_=xt[s, :, :, co + c0 : co + c0 + cw].rearrange(
                                "h p w -> p h w"
                            ),
                        )

                    def xsl(h, c0, cw):
                        return xs_all[:, h * W + c0 : h * W + c0 + cw]

                    acts = [
                        apool.tile([128, W], bf16, tag=f"a{i}", name=f"a{i}_{jn}")
                        for i in range(IT)
                    ]

                    # Phase A: chunk-outer / i-inner, with all 22 w0/w1 slabs
                    # loaded during the first chunk pass and kept resident in
                    # SBUF (88KB/partition).  This spreads the x-chunk DMA
                    # demand over the whole phase instead of front-loading it,
                    # and each slab is still loaded exactly once per job.
                    slabs = {}

                    def load_slab_pair(i, eng=None):
                        w0s = wp.tile([128, H], bf16, tag=f"w0_{i}", bufs=1)
                        (eng or nc.sync).dma_start(out=w0s, in_=w0t[s, i])
                        w1s = wp.tile([128, H], bf16, tag=f"w1_{i}", bufs=1)
                        (eng or nc.sync).dma_start(out=w1s, in_=w1t[s, i])
                        slabs[i] = (w0s, w1s)

                    # gate order: w0 slab + x chunk 0 (the first matmul
                    # group's inputs) ahead of w1 in the DMA queue
                    w0s = wp.tile([128, H], bf16, tag="w0_0", bufs=1)
                    nc.sync.dma_start(out=w0s, in_=w0t[s, 0])
                    load_x_chunk(0)
                    w1s = wp.tile([128, H], bf16, tag="w1_0", bufs=1)
                    nc.sync.dma_start(out=w1s, in_=w1t[s, 0])
                    slabs[0] = (w0s, w1s)
                    for ci, (c0, cw) in enumerate(chunks):
                        for i in range(IT):
                            if ci == 0:
                                # prefetch next slab pair one group ahead
                                if i + 1 < IT:
                                    load_slab_pair(i + 1)
                                if i == 2 and len(chunks) > 1:
                                    load_x_chunk(1)
                                if i == 6:
                                    for cj in range(2, len(chunks)):
                                        load_x_chunk(cj)
                            w0s, w1s = slabs[i]
                            g = ps.tile([128, 512], f32, tag="g")
                            for h in range(HT):
                                nc.tensor.matmul(
                                    g[:, :cw],
                                    w0s[:, ts(h, 128)],
                                    xsl(h, c0, cw),
                                    start=(h == 0),
                                    stop=(h == HT - 1),
                                )
                            u = ps.tile([128, 512], f32, tag="u")
                            for h in range(HT):
                                nc.tensor.matmul(
                                    u[:, :cw],
                                    w1s[:, ts(h, 128)],
                                    xsl(h, c0, cw),
                                    start=(h == 0),
                                    stop=(h == HT - 1),
                                )
                            a_sl = acts[i][:, c0 : c0 + cw]
                            nc.scalar.activation(a_sl, g[:, :cw], act_fn)
                            nc.vector.tensor_mul(a_sl, a_sl, u[:, :cw])

                    # Phase B: down projection, per h-tile.
                    for h in range(HT):
                        w2s = wp.tile([128, I], bf16, tag="w2", bufs=3)
                        nc.sync.dma_start(out=w2s, in_=w2t[s, h])
                        yc = yp.tile([128, W], bf16, tag="y", bufs=3)
                        for c0, cw in chunks:
                            o = ps.tile([128, 512], f32, tag="o", bufs=4)
                            for i in range(IT):
                                nc.tensor.matmul(
                                    o[:, :cw],
                                    w2s[:, ts(i, 128)],
                                    acts[i][:, c0 : c0 + cw],
                                    start=(i == 0),
                                    stop=(i == IT - 1),
                                )
                            nc.vector.tensor_copy(yc[:, c0 : c0 + cw], o[:, :cw])
                            if jn == len(jobs) - 1 and h == HT - 1:
                                # final h-tile: per-chunk writes shrink the tail
                                nc.scalar.dma_start(
                                    out=yt[s, h, :, co + c0 : co + c0 + cw],
                                    in_=yc[:, c0 : c0 + cw],
                                )
                        # one batched y write per h-tile, issued from the
                        # Scalar engine's HW-DGE queue so its CAST-waits never
                        # head-of-line block the input loads on the Sync queue.
                        if not (jn == len(jobs) - 1 and h == HT - 1):
                            nc.scalar.dma_start(
                                out=yt[s, h, :, co : co + W], in_=yc[:, :W]
                            )
    nc.finalize()
    return nc


def _get_built(jobs, CT):
    key = (tuple(jobs), CT)
    if key not in _BUILT:
        _BUILT[key] = _build(tuple(jobs), CT)
    return _BUILT[key]


def _dequant(w, s):
    """w: [E, O, Iin], s: [E, O, Iin//128] -> dequantized [E, O, Iin]."""
    e, o, iin = w.shape
    return (w.reshape(e, o, iin // BS, BS) * s[..., None]).reshape(e, o, iin)


def _bf16(a):
    import ml_dtypes

    return np.ascontiguousarray(a.astype(ml_dtypes.bfloat16))


def kernel(**inputs):
    global LAST_RESULTS
    x = np.ascontiguousarray(np.asarray(inputs["x"], dtype=np.float32))
    sel = np.asarray(inputs["selected_experts"])
    w0 = np.asarray(inputs["w0"], dtype=np.float32)
    s0 = np.asarray(inputs["s0"], dtype=np.float32)
    w1 = np.asarray(inputs["w1"], dtype=np.float32)
    s1 = np.asarray(inputs["s1"], dtype=np.float32)
    w2 = np.asarray(inputs["w2"], dtype=np.float32)
    s2 = np.asarray(inputs["s2"], dtype=np.float32)

    t, k = sel.shape
    assert (t, k) == (T, TOPK) and x.shape == (T, H)

    # ---- host-side dispatch: unique tokens per expert ----
    pos = np.full((E, T), -1, dtype=np.int32)
    cols = []
    for e in range(E):
        toks = np.nonzero((sel == e).any(axis=1))[0]
        cols.append(toks)
        pos[e, toks] = np.arange(len(toks), dtype=np.int32)
    counts = np.array([len(c) for c in cols])

    # Assign experts to (core, slot): slot 0 holds the 8 largest experts,
    # slot 1 the 8 smallest, so each slot's padded width is only the max of
    # its own rank group.  expert_of[s][c] = expert on core c, slot s.
    order = np.argsort(-counts, kind="stable")
    expert_of = [list(order[:NCORES]), list(order[NCORES:])]

    def align8(v):
        return max(256, -(-v // 8) * 8)

    slot_w = [align8(int(counts[expert_of[s]].max())) for s in range(2)]

    if max(slot_w) <= MAX_W:
        jobs = tuple((s, 0, slot_w[s]) for s in range(2))
        CT = max(slot_w)
    else:
        # fallback: uniform width, multiple column windows per slot
        cmax = int(counts.max())
        passes = max(1, math.ceil(cmax / MAX_W))
        W = align8(math.ceil(cmax / passes))
        CT = W * passes
        jobs = tuple((s, cp * W, W) for s in range(2) for cp in range(passes))

    # ---- dequantize + slab-major weight prep (host) ----
    W0 = _dequant(w0, s0)  # [E, I, H]
    W1 = _dequant(w1, s1)  # [E, I, H]
    W2 = _dequant(w2, s2)  # [E, H, I]
    # w01 slab-major: [E, IT, 128p, H] with [it, p, hb*128+j] = W[it*128+j, hb*128+p]
    w0n = _bf16(W0.reshape(E, IT, 128, HT, 128).transpose(0, 1, 4, 3, 2).reshape(E, IT, 128, H))
    w1n = _bf16(W1.reshape(E, IT, 128, HT, 128).transpose(0, 1, 4, 3, 2).reshape(E, IT, 128, H))
    # w2 slab-major: [E, HT, 128p, I] with [ht, p, it*128+j] = W2[ht*128+j, it*128+p]
    w2n = _bf16(W2.reshape(E, HT, 128, IT, 128).transpose(0, 1, 4, 3, 2).reshape(E, HT, 128, I))

    in_maps = []
    for c in range(NCORES):
        pair = [expert_of[0][c], expert_of[1][c]]
        xt_c = np.zeros((2, H, CT), dtype=np.float32)
        for s, e in enumerate(pair):
            n = len(cols[e])
            if n:
                xt_c[s, :, :n] = x[cols[e]].T
        in_maps.append(
            {
                "xt": _bf16(xt_c.reshape(2, HT, 128, CT)),
                "w0t": np.ascontiguousarray(w0n[pair]),
                "w1t": np.ascontiguousarray(w1n[pair]),
                "w2t": np.ascontiguousarray(w2n[pair]),
            }
        )

    nc = _get_built(jobs, CT)
    from concourse.bass_utils import run_bass_kernel_spmd

    res = run_bass_kernel_spmd(nc, in_maps, list(range(NCORES)))
    LAST_RESULTS = res

    # Y[e] = [H, CT] for expert e
    Y = np.empty((E, H, CT), dtype=np.float32)
    for c in range(NCORES):
        yt_c = np.asarray(res.results[c]["yt"]).astype(np.float32).reshape(2, H, CT)
        Y[expert_of[0][c]] = yt_c[0]
        Y[expert_of[1][c]] = yt_c[1]

    # ---- scatter back to [T, K, H] ----
    e_flat = sel.reshape(-1).astype(np.int64)
    t_flat = np.repeat(np.arange(T, dtype=np.int64), TOPK)
    p_flat = pos[e_flat, t_flat]
    out = Y[e_flat, :, p_flat]  # [T*K, H]
    return np.ascontiguousarray(out.reshape(T, TOPK, H), dtype=np.float32)
